# revision 2
# baseline (speedup 1.0000x reference)
"""Multi-head contextual biased attention on 8 Trainium2 NeuronCores.

Sharding: data-parallel over batch (B=2) x tensor-parallel over heads
(16 heads -> 4 per core). Each core computes Q/K/V projections for its
4 heads, streaming-softmax attention with the periodic ALiBi-style bias
applied as a precomputed multiplicative table (exp(bias) folded in after
exp(scores)), and a partial output projection. The host sums the 4
partial output projections per batch element and adds the bias bo.

Device layout notes:
  - scores are computed transposed (S^T[j, i], context j on partitions) so
    the P@V contraction can run with V as the stationary operand; a ones
    column appended to V yields softmax denominators in the same matmul.
  - exp(bias*head_scale) depends only on (j - i), so it is stored as one
    skewed per-partition sequence eb[p, t] = g(p + t - T0) and addressed
    per tile with a step -1 access pattern; the multiply runs on the DVE
    in bf16 2x mode.
"""

import numpy as np
import ml_dtypes
from contextlib import ExitStack

import concourse.bass as bass
import concourse.tile as tile
from concourse import bacc, mybir
from concourse.bass_utils import run_bass_kernel_spmd

bf16 = ml_dtypes.bfloat16
F32 = mybir.dt.float32
BF16 = mybir.dt.bfloat16
Exp = mybir.ActivationFunctionType.Exp

B, T, D = 2, 2048, 1024
NH, DH = 16, 64          # global heads, head dim
HL = 4                   # heads per core
KC = D // 128            # contraction chunks
PERIOD = 30
T0 = 2049                # odd skew origin (odd => step -1 APs stay 4B-aligned)
EBL = 3972               # skew table length


def _build_kernel(ctx, tc, y_d, xt_d, ct_d, wq_d, wk_d, wv_d, wo_d, eb_d):
    nc = tc.nc

    const = ctx.enter_context(tc.tile_pool(name="const", bufs=1))
    data = ctx.enter_context(tc.tile_pool(name="data", bufs=1))

    eb_sb = const.tile([128, HL, EBL], BF16)
    nc.sync.dma_start(eb_sb[:], eb_d[:])
    wq_sb = const.tile([128, KC, 256], BF16)
    nc.sync.dma_start(wq_sb[:], wq_d[:])
    wk_sb = const.tile([128, KC, 256], BF16)
    nc.sync.dma_start(wk_sb[:], wk_d[:])
    wv_sb = const.tile([128, KC, 256], BF16)
    nc.sync.dma_start(wv_sb[:], wv_d[:])
    wo_sb = const.tile([64, HL, D], BF16)
    nc.sync.dma_start(wo_sb[:], wo_d[:])

    ct_sb = data.tile([128, KC, T], BF16)
    for k in range(KC):
        nc.sync.dma_start(ct_sb[:, k, :], ct_d[:, k, :])

    qT_sb = data.tile([128, 2, T], BF16)
    kT_sb = data.tile([128, 2, T], BF16)
    v_sb = data.tile([128, 16, HL, 65], BF16)
    o_sb = data.tile([64, HL, T], BF16)
    nc.vector.memset(v_sb[:, :, :, 64:65], 1.0)

    # ---- Phase 1: projections ----
    with tc.tile_pool(name="pps", bufs=4, space="PSUM") as pps, \
         tc.tile_pool(name="xs", bufs=10) as xs:
        # q^T[d, i] (scale folded into wq on host)
        for it in range(4):
            xts = []
            for k in range(KC):
                t = xs.tile([128, 512], BF16, tag="xt", name=f"xt_{it}_{k}")
                nc.sync.dma_start(t[:], xt_d[:, k, it * 512:(it + 1) * 512])
                xts.append(t)
            for m in range(2):
                ps = pps.tile([128, 512], F32, tag="mm", name=f"qps_{it}_{m}")
                for k in range(KC):
                    nc.tensor.matmul(ps[:], lhsT=wq_sb[:, k, m * 128:(m + 1) * 128],
                                     rhs=xts[k][:], start=(k == 0), stop=(k == KC - 1))
                nc.vector.tensor_copy(qT_sb[:, m, it * 512:(it + 1) * 512], ps[:])
        # k^T[d, j]
        for it in range(4):
            for m in range(2):
                ps = pps.tile([128, 512], F32, tag="mm", name=f"kps_{it}_{m}")
                for k in range(KC):
                    nc.tensor.matmul(ps[:], lhsT=wk_sb[:, k, m * 128:(m + 1) * 128],
                                     rhs=ct_sb[:, k, it * 512:(it + 1) * 512],
                                     start=(k == 0), stop=(k == KC - 1))
                nc.vector.tensor_copy(kT_sb[:, m, it * 512:(it + 1) * 512], ps[:])
        # v[j, d] in per-head stationary layout
        for jt in range(16):
            ps = pps.tile([128, 512], F32, tag="mm", name=f"vps_{jt}")
            for k in range(KC):
                nc.tensor.matmul(ps[:, 0:256], lhsT=ct_sb[:, k, jt * 128:(jt + 1) * 128],
                                 rhs=wv_sb[:, k, :], start=(k == 0), stop=(k == KC - 1))
            nc.vector.tensor_copy(
                v_sb[:, jt, :, 0:64],
                ps[:, 0:256].rearrange("p (h d) -> p h d", h=HL))

    # ---- Phase 2: attention per head ----
    with tc.tile_pool(name="sps", bufs=2, space="PSUM") as sps, \
         tc.tile_pool(name="pvs", bufs=1, space="PSUM") as pvs, \
         tc.tile_pool(name="pp", bufs=3) as pp, \
         tc.tile_pool(name="nrm", bufs=2) as nrm:
        for h in range(HL):
            m = h // 2
            hp = (h % 2) * 64
            pv = pvs.tile([65, T], F32, tag="pv", name=f"pv_{h}")
            for jt in range(16):
                pt = pp.tile([128, T], BF16, tag="p", name=f"p_{h}_{jt}")
                for half in range(2):
                    sp = sps.tile([128, 1024], F32, tag="s", name=f"s_{h}_{jt}_{half}")
                    for it2 in range(2):
                        i0 = half * 1024 + it2 * 512
                        nc.tensor.matmul(sp[:, it2 * 512:(it2 + 1) * 512],
                                         lhsT=kT_sb[hp:hp + 64, m, jt * 128:(jt + 1) * 128],
                                         rhs=qT_sb[hp:hp + 64, m, i0:i0 + 512],
                                         start=True, stop=True)
                    nc.scalar.activation(pt[:, half * 1024:(half + 1) * 1024], sp[:], Exp)
                idx0 = T0 + jt * 128
                nc.vector.tensor_mul(pt[:], pt[:], eb_sb[:, h, idx0:idx0 - T:-1])
                for it in range(4):
                    nc.tensor.matmul(pv[:, it * 512:(it + 1) * 512],
                                     lhsT=v_sb[:, jt, h, :],
                                     rhs=pt[:, it * 512:(it + 1) * 512],
                                     start=(jt == 0), stop=(jt == 15))
            # normalize: o[d, i] = pv[d, i] / pv[64, i]
            rs = nrm.tile([1, T], F32, tag="rs", name=f"rs_{h}")
            nc.scalar.copy(rs[:], pv[64:65, :])
            rsq = nrm.tile([128, 16], F32, tag="rsq", name=f"rsq_{h}")
            nc.sync.dma_start(rsq[:], rs[:])
            rsr = nrm.tile([128, 16], F32, tag="rsr", name=f"rsr_{h}")
            nc.vector.reciprocal(rsr[:], rsq[:])
            rsf = nrm.tile([1, T], F32, tag="rsf", name=f"rsf_{h}")
            nc.sync.dma_start(rsf[:], rsr[:])
            rsb = nrm.tile([64, T], F32, tag="rsb", name=f"rsb_{h}")
            nc.gpsimd.partition_broadcast(rsb[:], rsf[:], channels=64)
            nc.vector.tensor_mul(o_sb[:, h, :], pv[0:64, :], rsb[:])

    # ---- Phase 3: output projection (partial; host sums across head-groups) ----
    with tc.tile_pool(name="yps", bufs=3, space="PSUM") as yps, \
         tc.tile_pool(name="yo", bufs=3) as yo:
        for ic in range(16):
            for mt in range(2):
                ps = yps.tile([128, 512], F32, tag="y", name=f"yps_{ic}_{mt}")
                for h in range(HL):
                    nc.tensor.matmul(ps[:], lhsT=o_sb[:, h, ic * 128:(ic + 1) * 128],
                                     rhs=wo_sb[:, h, mt * 512:(mt + 1) * 512],
                                     start=(h == 0), stop=(h == HL - 1))
                yt = yo.tile([128, 512], F32, tag="yt", name=f"yt_{ic}_{mt}")
                nc.scalar.copy(yt[:], ps[:])
                nc.sync.dma_start(y_d[ic * 128:(ic + 1) * 128, mt * 512:(mt + 1) * 512],
                                  yt[:])


_NC = None


def build_nc():
    global _NC
    if _NC is not None:
        return _NC
    nc = bacc.Bacc("TRN2", target_bir_lowering=False, debug=False, num_devices=8)
    xt_d = nc.dram_tensor("xt", [128, KC, T], BF16, kind="ExternalInput").ap()
    ct_d = nc.dram_tensor("ct", [128, KC, T], BF16, kind="ExternalInput").ap()
    wq_d = nc.dram_tensor("wq", [128, KC, 256], BF16, kind="ExternalInput").ap()
    wk_d = nc.dram_tensor("wk", [128, KC, 256], BF16, kind="ExternalInput").ap()
    wv_d = nc.dram_tensor("wv", [128, KC, 256], BF16, kind="ExternalInput").ap()
    wo_d = nc.dram_tensor("wo", [64, HL, D], BF16, kind="ExternalInput").ap()
    eb_d = nc.dram_tensor("eb", [128, HL, EBL], BF16, kind="ExternalInput").ap()
    y_d = nc.dram_tensor("y", [T, D], F32, kind="ExternalOutput").ap()

    with tile.TileContext(nc) as tc, ExitStack() as ctx:
        _build_kernel(ctx, tc, y_d, xt_d, ct_d, wq_d, wk_d, wv_d, wo_d, eb_d)
    nc.compile()
    _NC = nc
    return nc


def _to_chunked(mat_t, cols):
    """[D, cols] -> [128, KC, cols] with partition dim first."""
    return np.ascontiguousarray(
        mat_t.reshape(KC, 128, cols).transpose(1, 0, 2)).astype(bf16)


def make_in_maps(x, context, Wq, Wk, Wv, Wo):
    scale = np.float32(1.0 / np.sqrt(DH))
    # exp-bias skew tables per global head
    p = np.arange(128, dtype=np.int64)[:, None]
    t = np.arange(EBL, dtype=np.int64)[None, :]
    dist = np.abs(p + t - T0) // PERIOD          # [128, EBL]
    in_maps = []
    for c in range(8):
        b = c // 4
        h0 = (c % 4) * HL
        rows = slice(h0 * DH, (h0 + HL) * DH)
        xt = np.ascontiguousarray(
            x[b].T.reshape(KC, 128, T).transpose(1, 0, 2)).astype(bf16)
        ct = np.ascontiguousarray(
            context[b].T.reshape(KC, 128, T).transpose(1, 0, 2)).astype(bf16)
        wq = _to_chunked(np.ascontiguousarray((Wq[rows] * scale).T), 256)
        wk = _to_chunked(np.ascontiguousarray(Wk[rows].T), 256)
        wv = _to_chunked(np.ascontiguousarray(Wv[rows].T), 256)
        wo = np.ascontiguousarray(
            Wo[:, rows].T.reshape(HL, DH, D).transpose(1, 0, 2)).astype(bf16)
        eb = np.empty((128, HL, EBL), dtype=bf16)
        for hl in range(HL):
            hs = 2.0 ** (-(h0 + hl + 1))
            eb[:, hl, :] = np.exp(-hs * dist).astype(bf16)
        in_maps.append({"xt": xt, "ct": ct, "wq": wq, "wk": wk, "wv": wv,
                        "wo": wo, "eb": np.ascontiguousarray(eb)})
    return in_maps


def kernel(x, context, Wq, Wk, Wv, Wo, bo, _collect=None):
    x = np.asarray(x, dtype=np.float32)
    context = np.asarray(context, dtype=np.float32)
    Wq = np.asarray(Wq, dtype=np.float32)
    Wk = np.asarray(Wk, dtype=np.float32)
    Wv = np.asarray(Wv, dtype=np.float32)
    Wo = np.asarray(Wo, dtype=np.float32)
    bo = np.asarray(bo, dtype=np.float32)

    nc = build_nc()
    in_maps = make_in_maps(x, context, Wq, Wk, Wv, Wo)
    res = run_bass_kernel_spmd(nc, in_maps, list(range(8)))
    if _collect is not None:
        _collect.append(res)

    out = np.empty((B, T, D), dtype=np.float32)
    for b in range(2):
        acc = res.results[4 * b]["y"].astype(np.float32).copy()
        for c in range(4 * b + 1, 4 * b + 4):
            acc += res.results[c]["y"]
        out[b] = acc + bo[None, :]
    return out


# revision 7
# speedup vs baseline: 1.0361x; 1.0361x over previous
"""Multi-head contextual biased attention on 8 Trainium2 NeuronCores.

Sharding: data-parallel over batch (B=2) x tensor-parallel over heads
(16 heads -> 4 per core). Each core computes Q/K/V projections for its
4 heads, streaming-softmax attention with the periodic ALiBi-style bias
applied as a precomputed multiplicative table (exp(bias) folded in after
exp(scores)), and a partial output projection. The host sums the 4
partial output projections per batch element and adds the bias bo.

Device layout notes:
  - scores are computed transposed (S^T[j, i], context j on partitions) so
    the P@V contraction can run with V as the stationary operand; a ones
    column appended to V yields softmax denominators in the same matmul.
  - exp(bias*head_scale) depends only on (j - i), so it is stored as one
    skewed per-partition sequence eb[p, t] = g(p + t - T0) and addressed
    per tile with a step -1 access pattern; the multiply runs on the DVE
    in bf16 2x mode.
"""

import numpy as np
import ml_dtypes
from contextlib import ExitStack

import concourse.bass as bass
import concourse.tile as tile
from concourse import bacc, mybir
from concourse.bass_utils import run_bass_kernel_spmd

bf16 = ml_dtypes.bfloat16
F32 = mybir.dt.float32
BF16 = mybir.dt.bfloat16
Exp = mybir.ActivationFunctionType.Exp

B, T, D = 2, 2048, 1024
NH, DH = 16, 64          # global heads, head dim
HL = 4                   # heads per core
KC = D // 128            # contraction chunks
PERIOD = 30
T0 = 2049                # odd skew origin (odd => step -1 APs stay 4B-aligned)
EBL = 3972               # skew table length


def _build_kernel(ctx, tc, y_d, xt_d, ct_d, wq_d, wk_d, wv_d, wo_d, eb_d):
    nc = tc.nc

    const = ctx.enter_context(tc.tile_pool(name="const", bufs=1))
    data = ctx.enter_context(tc.tile_pool(name="data", bufs=1))

    # DMA issue order matters: the q-projection only needs wq + the first x
    # tiles, so those go first; bulk loads (eb, wo) are issued later so they
    # overlap compute instead of delaying it.
    wq_sb = const.tile([128, KC, 256], BF16)
    nc.sync.dma_start(wq_sb[:], wq_d[:])
    wk_sb = const.tile([128, KC, 256], BF16)
    wv_sb = const.tile([128, KC, 256], BF16)
    wo_sb = const.tile([128, 2, D], BF16)
    eb_sb = const.tile([128, HL, EBL], BF16)
    ct_sb = data.tile([128, KC, T], BF16)

    qT_sb = data.tile([128, 2, T], BF16)
    kT_sb = data.tile([128, 2, T], BF16)
    v_sb = data.tile([128, 16, HL, 65], BF16)
    o2_sb = data.tile([128, 2, T], BF16)
    nc.vector.memset(v_sb[:, :, :, 64:65], 1.0)

    # ---- Phase 1: projections ----
    with tc.tile_pool(name="pps", bufs=4, space="PSUM") as pps, \
         tc.tile_pool(name="xs", bufs=10) as xs:
        # q^T[d, i] (scale folded into wq on host)
        for it in range(4):
            xts = []
            for k in range(KC):
                t = xs.tile([128, 512], BF16, tag="xt", name=f"xt_{it}_{k}")
                nc.sync.dma_start(t[:], xt_d[:, k, it * 512:(it + 1) * 512])
                xts.append(t)
            if it == 0:
                # background loads, issued after the critical q-path DMAs
                for k in range(KC):
                    nc.sync.dma_start(ct_sb[:, k, :], ct_d[:, k, :])
                nc.sync.dma_start(wk_sb[:], wk_d[:])
                nc.sync.dma_start(wv_sb[:], wv_d[:])
                nc.sync.dma_start(eb_sb[:], eb_d[:])
                nc.sync.dma_start(wo_sb[:], wo_d[:])
            for m in range(2):
                ps = pps.tile([128, 512], F32, tag="mm", name=f"qps_{it}_{m}")
                for k in range(KC):
                    nc.tensor.matmul(ps[:], lhsT=wq_sb[:, k, m * 128:(m + 1) * 128],
                                     rhs=xts[k][:], start=(k == 0), stop=(k == KC - 1))
                nc.vector.tensor_copy(qT_sb[:, m, it * 512:(it + 1) * 512], ps[:])
        # k^T[d, j]
        for it in range(4):
            for m in range(2):
                ps = pps.tile([128, 512], F32, tag="mm", name=f"kps_{it}_{m}")
                for k in range(KC):
                    nc.tensor.matmul(ps[:], lhsT=wk_sb[:, k, m * 128:(m + 1) * 128],
                                     rhs=ct_sb[:, k, it * 512:(it + 1) * 512],
                                     start=(k == 0), stop=(k == KC - 1))
                nc.vector.tensor_copy(kT_sb[:, m, it * 512:(it + 1) * 512], ps[:])
        # v[j, d] in per-head stationary layout
        for jt in range(16):
            ps = pps.tile([128, 512], F32, tag="mm", name=f"vps_{jt}")
            for k in range(KC):
                nc.tensor.matmul(ps[:, 0:256], lhsT=ct_sb[:, k, jt * 128:(jt + 1) * 128],
                                 rhs=wv_sb[:, k, :], start=(k == 0), stop=(k == KC - 1))
            nc.vector.tensor_copy(
                v_sb[:, jt, :, 0:64],
                ps[:, 0:256].rearrange("p (h d) -> p h d", h=HL))

    # ---- Phase 2: attention per head ----
    with tc.tile_pool(name="sps", bufs=2, space="PSUM") as sps, \
         tc.tile_pool(name="pvs", bufs=1, space="PSUM") as pvs, \
         tc.tile_pool(name="pp", bufs=3) as pp, \
         tc.tile_pool(name="nrm", bufs=2) as nrm:
        for h in range(HL):
            m = h // 2
            hp = (h % 2) * 64
            pv = pvs.tile([65, T], F32, tag="pv", name=f"pv_{h}")
            for jt in range(16):
                pt = pp.tile([128, T], BF16, tag="p", name=f"p_{h}_{jt}")
                idx0 = T0 + jt * 128
                for half in range(2):
                    sp = sps.tile([128, 1024], F32, tag="s", name=f"s_{h}_{jt}_{half}")
                    for it2 in range(2):
                        i0 = half * 1024 + it2 * 512
                        nc.tensor.matmul(sp[:, it2 * 512:(it2 + 1) * 512],
                                         lhsT=kT_sb[hp:hp + 64, m, jt * 128:(jt + 1) * 128],
                                         rhs=qT_sb[hp:hp + 64, m, i0:i0 + 512],
                                         start=True, stop=True)
                    hsl = slice(half * 1024, (half + 1) * 1024)
                    nc.scalar.activation(pt[:, hsl], sp[:], Exp)
                    nc.vector.tensor_mul(pt[:, hsl], pt[:, hsl],
                                         eb_sb[:, h, idx0 - half * 1024:
                                               idx0 - (half + 1) * 1024:-1])
                    for it in range(2):
                        i0 = half * 1024 + it * 512
                        nc.tensor.matmul(pv[:, i0:i0 + 512],
                                         lhsT=v_sb[:, jt, h, :],
                                         rhs=pt[:, i0:i0 + 512],
                                         start=(jt == 0), stop=(jt == 15))
            # fast psum release: copy pv -> sbuf (split across ACT/DVE), then
            # run the (slow) normalization chain off the critical path.
            pvf = nrm.tile([65, T], F32, tag="pvf", name=f"pvf_{h}")
            nc.scalar.copy(pvf[:, 0:1024], pv[:, 0:1024])
            nc.vector.tensor_copy(pvf[:, 1024:T], pv[:, 1024:T])
            rsq = nrm.tile([128, 16], F32, tag="rsq", name=f"rsq_{h}")
            nc.sync.dma_start(rsq[:], pvf[64:65, :])
            rsr = nrm.tile([128, 16], F32, tag="rsr", name=f"rsr_{h}")
            nc.vector.reciprocal(rsr[:], rsq[:])
            rsf = nrm.tile([1, T], F32, tag="rsf", name=f"rsf_{h}")
            nc.sync.dma_start(rsf[:], rsr[:])
            rsb = nrm.tile([64, T], F32, tag="rsb", name=f"rsb_{h}")
            nc.gpsimd.partition_broadcast(rsb[:], rsf[:], channels=64)
            # normalized heads land in o2 [128, 2, T]: even head -> partitions
            # 0..63 directly; odd head -> staging tile then DMA to 64..127.
            if h % 2 == 0:
                nc.vector.tensor_mul(o2_sb[0:64, m, :], pvf[0:64, :], rsb[:])
            else:
                otmp = nrm.tile([64, T], BF16, tag="otmp", name=f"otmp_{h}")
                nc.vector.tensor_mul(otmp[:], pvf[0:64, :], rsb[:])
                nc.sync.dma_start(o2_sb[64:128, m, :], otmp[:])

    # ---- Phase 3: output projection (partial; host sums across head-groups) ----
    with tc.tile_pool(name="yps", bufs=3, space="PSUM") as yps, \
         tc.tile_pool(name="yo", bufs=3) as yo:
        for ic in range(16):
            for mt in range(2):
                ps = yps.tile([128, 512], F32, tag="y", name=f"yps_{ic}_{mt}")
                for m in range(2):
                    nc.tensor.matmul(ps[:], lhsT=o2_sb[:, m, ic * 128:(ic + 1) * 128],
                                     rhs=wo_sb[:, m, mt * 512:(mt + 1) * 512],
                                     start=(m == 0), stop=(m == 1))
                yt = yo.tile([128, 512], F32, tag="yt", name=f"yt_{ic}_{mt}")
                nc.scalar.copy(yt[:], ps[:])
                nc.sync.dma_start(y_d[ic * 128:(ic + 1) * 128, mt * 512:(mt + 1) * 512],
                                  yt[:])


_NC = None


def build_nc():
    global _NC
    if _NC is not None:
        return _NC
    nc = bacc.Bacc("TRN2", target_bir_lowering=False, debug=False, num_devices=8)
    xt_d = nc.dram_tensor("xt", [128, KC, T], BF16, kind="ExternalInput").ap()
    ct_d = nc.dram_tensor("ct", [128, KC, T], BF16, kind="ExternalInput").ap()
    wq_d = nc.dram_tensor("wq", [128, KC, 256], BF16, kind="ExternalInput").ap()
    wk_d = nc.dram_tensor("wk", [128, KC, 256], BF16, kind="ExternalInput").ap()
    wv_d = nc.dram_tensor("wv", [128, KC, 256], BF16, kind="ExternalInput").ap()
    wo_d = nc.dram_tensor("wo", [128, 2, D], BF16, kind="ExternalInput").ap()
    eb_d = nc.dram_tensor("eb", [128, HL, EBL], BF16, kind="ExternalInput").ap()
    y_d = nc.dram_tensor("y", [T, D], F32, kind="ExternalOutput").ap()

    with tile.TileContext(nc) as tc, ExitStack() as ctx:
        _build_kernel(ctx, tc, y_d, xt_d, ct_d, wq_d, wk_d, wv_d, wo_d, eb_d)
    nc.compile()
    _NC = nc
    return nc


def _to_chunked(mat_t, cols):
    """[D, cols] -> [128, KC, cols] with partition dim first."""
    return np.ascontiguousarray(
        mat_t.reshape(KC, 128, cols).transpose(1, 0, 2)).astype(bf16)


def make_in_maps(x, context, Wq, Wk, Wv, Wo):
    scale = np.float32(1.0 / np.sqrt(DH))
    # exp-bias skew tables per global head
    p = np.arange(128, dtype=np.int64)[:, None]
    t = np.arange(EBL, dtype=np.int64)[None, :]
    dist = np.abs(p + t - T0) // PERIOD          # [128, EBL]
    in_maps = []
    for c in range(8):
        b = c // 4
        h0 = (c % 4) * HL
        rows = slice(h0 * DH, (h0 + HL) * DH)
        xt = np.ascontiguousarray(
            x[b].T.reshape(KC, 128, T).transpose(1, 0, 2)).astype(bf16)
        ct = np.ascontiguousarray(
            context[b].T.reshape(KC, 128, T).transpose(1, 0, 2)).astype(bf16)
        wq = _to_chunked(np.ascontiguousarray((Wq[rows] * scale).T), 256)
        wk = _to_chunked(np.ascontiguousarray(Wk[rows].T), 256)
        wv = _to_chunked(np.ascontiguousarray(Wv[rows].T), 256)
        wo = np.ascontiguousarray(
            Wo[:, rows].T.reshape(2, 128, D).transpose(1, 0, 2)).astype(bf16)
        eb = np.empty((128, HL, EBL), dtype=bf16)
        for hl in range(HL):
            hs = 2.0 ** (-(h0 + hl + 1))
            eb[:, hl, :] = np.exp(-hs * dist).astype(bf16)
        in_maps.append({"xt": xt, "ct": ct, "wq": wq, "wk": wk, "wv": wv,
                        "wo": wo, "eb": np.ascontiguousarray(eb)})
    return in_maps


def kernel(x, context, Wq, Wk, Wv, Wo, bo, _collect=None):
    x = np.asarray(x, dtype=np.float32)
    context = np.asarray(context, dtype=np.float32)
    Wq = np.asarray(Wq, dtype=np.float32)
    Wk = np.asarray(Wk, dtype=np.float32)
    Wv = np.asarray(Wv, dtype=np.float32)
    Wo = np.asarray(Wo, dtype=np.float32)
    bo = np.asarray(bo, dtype=np.float32)

    nc = build_nc()
    in_maps = make_in_maps(x, context, Wq, Wk, Wv, Wo)
    res = run_bass_kernel_spmd(nc, in_maps, list(range(8)))
    if _collect is not None:
        _collect.append(res)

    out = np.empty((B, T, D), dtype=np.float32)
    for b in range(2):
        acc = res.results[4 * b]["y"].astype(np.float32).copy()
        for c in range(4 * b + 1, 4 * b + 4):
            acc += res.results[c]["y"]
        out[b] = acc + bo[None, :]
    return out


# revision 12
# speedup vs baseline: 1.1175x; 1.0786x over previous
"""Multi-head contextual biased attention on 8 Trainium2 NeuronCores.

Sharding: data-parallel over batch (B=2) x tensor-parallel over heads
(16 heads -> 4 per core). Each core computes Q/K/V projections for its
4 heads, streaming-softmax attention with the periodic ALiBi-style bias
applied as a precomputed multiplicative table (exp(bias) folded in after
exp(scores)), and a partial output projection. The host sums the 4
partial output projections per batch element and adds the bias bo.

Device layout notes:
  - scores are computed transposed (S^T[j, i], context j on partitions) so
    the P@V contraction can run with V as the stationary operand; a ones
    column appended to V yields softmax denominators in the same matmul.
  - exp(bias*head_scale) depends only on (j - i), so it is stored as one
    skewed per-partition sequence eb[p, t] = g(p + t - T0) and addressed
    per tile with a step -1 access pattern; the multiply runs on the DVE
    in bf16 2x mode.
"""

import numpy as np
import ml_dtypes
from contextlib import ExitStack

import concourse.bass as bass
import concourse.tile as tile
from concourse import bacc, mybir
from concourse.bass_utils import run_bass_kernel_spmd

bf16 = ml_dtypes.bfloat16
F32 = mybir.dt.float32
BF16 = mybir.dt.bfloat16
Exp = mybir.ActivationFunctionType.Exp

B, T, D = 2, 2048, 1024
NH, DH = 16, 64          # global heads, head dim
HL = 4                   # heads per core
KC = D // 128            # contraction chunks
PERIOD = 30
T0 = 2049                # odd skew origin (odd => step -1 APs stay 4B-aligned)
EBL = 3972               # skew table length


def _build_kernel(ctx, tc, y_d, xt_d, ct_d, wq_d, wk_d, wv_d, wo_d, eb_d):
    nc = tc.nc

    const = ctx.enter_context(tc.tile_pool(name="const", bufs=1))
    data = ctx.enter_context(tc.tile_pool(name="data", bufs=1))

    # DMA issue order matters: queues drain in issue order, so the q-path
    # inputs (wq, x) go first and bulk late-use loads (eb, wo) go last.
    wq_sb = const.tile([128, KC, 256], BF16)
    nc.sync.dma_start(wq_sb[:], wq_d[:])
    xt_sb = data.tile([128, KC, T], BF16)
    for k in range(KC):
        nc.sync.dma_start(xt_sb[:, k, :], xt_d[:, k, :])
    wk_sb = const.tile([128, KC, 256], BF16)
    nc.sync.dma_start(wk_sb[:], wk_d[:])
    wv_sb = const.tile([128, KC, 256], BF16)
    nc.sync.dma_start(wv_sb[:], wv_d[:])
    ct_sb = data.tile([128, KC, T], BF16)
    for k in range(KC):
        nc.sync.dma_start(ct_sb[:, k, :], ct_d[:, k, :])
    eb_sb = const.tile([128, HL, EBL], BF16)
    nc.sync.dma_start(eb_sb[:], eb_d[:])
    wo_sb = const.tile([128, 2, D], BF16)
    nc.sync.dma_start(wo_sb[:], wo_d[:])

    qT_sb = data.tile([128, 2, T], BF16)
    kT_sb = data.tile([128, 2, T], BF16)
    v_sb = data.tile([128, 16, HL, 65], BF16)
    o2_sb = data.tile([128, 2, T], BF16)
    nc.vector.memset(v_sb[:, :, :, 64:65], 1.0)

    # ---- Phase 1: projections ----
    with tc.tile_pool(name="pps", bufs=4, space="PSUM") as pps:
        # q^T[d, i] (scale folded into wq on host)
        for it in range(4):
            for m in range(2):
                ps = pps.tile([128, 512], F32, tag="mm", name=f"qps_{it}_{m}")
                for k in range(KC):
                    nc.tensor.matmul(ps[:], lhsT=wq_sb[:, k, m * 128:(m + 1) * 128],
                                     rhs=xt_sb[:, k, it * 512:(it + 1) * 512],
                                     start=(k == 0), stop=(k == KC - 1))
                nc.vector.tensor_copy(qT_sb[:, m, it * 512:(it + 1) * 512], ps[:])
        # k^T[d, j]
        for it in range(4):
            for m in range(2):
                ps = pps.tile([128, 512], F32, tag="mm", name=f"kps_{it}_{m}")
                for k in range(KC):
                    nc.tensor.matmul(ps[:], lhsT=wk_sb[:, k, m * 128:(m + 1) * 128],
                                     rhs=ct_sb[:, k, it * 512:(it + 1) * 512],
                                     start=(k == 0), stop=(k == KC - 1))
                nc.vector.tensor_copy(kT_sb[:, m, it * 512:(it + 1) * 512], ps[:])
        # v[j, d] in per-head stationary layout
        for jt in range(16):
            ps = pps.tile([128, 512], F32, tag="mm", name=f"vps_{jt}")
            for k in range(KC):
                nc.tensor.matmul(ps[:, 0:256], lhsT=ct_sb[:, k, jt * 128:(jt + 1) * 128],
                                 rhs=wv_sb[:, k, :], start=(k == 0), stop=(k == KC - 1))
            nc.vector.tensor_copy(
                v_sb[:, jt, :, 0:64],
                ps[:, 0:256].rearrange("p (h d) -> p h d", h=HL))

    # ---- Phase 2: attention per head ----
    # PV is software-pipelined one jt behind QK/exp/mult so the PE stream has
    # no dependency stall per jt (keeps PE dense -> HAM stays un-throttled).
    # Head order ends on an even head so the final (exposed) normalization
    # skips the odd-head staging DMA.
    with tc.tile_pool(name="sps", bufs=2, space="PSUM") as sps, \
         tc.tile_pool(name="pvs", bufs=1, space="PSUM") as pvs, \
         tc.tile_pool(name="pp", bufs=2) as pp, \
         tc.tile_pool(name="nrm", bufs=1) as nrm:
        for h in (0, 1, 3, 2):
            m = h // 2
            hp = (h % 2) * 64
            pv = pvs.tile([65, T], F32, tag="pv", name=f"pv_{h}")

            def emit_pv(jt, pt):
                for it in range(4):
                    nc.tensor.matmul(pv[:, it * 512:(it + 1) * 512],
                                     lhsT=v_sb[:, jt, h, :],
                                     rhs=pt[:, it * 512:(it + 1) * 512],
                                     start=(jt == 0), stop=(jt == 15))

            prev = None
            for jt in range(16):
                pt = pp.tile([128, T], BF16, tag="p", name=f"p_{h}_{jt}")
                idx0 = T0 + jt * 128
                for half in range(2):
                    sp = sps.tile([128, 1024], F32, tag="s", name=f"s_{h}_{jt}_{half}")
                    for it2 in range(2):
                        i0 = half * 1024 + it2 * 512
                        nc.tensor.matmul(sp[:, it2 * 512:(it2 + 1) * 512],
                                         lhsT=kT_sb[hp:hp + 64, m, jt * 128:(jt + 1) * 128],
                                         rhs=qT_sb[hp:hp + 64, m, i0:i0 + 512],
                                         start=True, stop=True)
                    hsl = slice(half * 1024, (half + 1) * 1024)
                    nc.scalar.activation(pt[:, hsl], sp[:], Exp)
                    nc.vector.tensor_mul(pt[:, hsl], pt[:, hsl],
                                         eb_sb[:, h, idx0 - half * 1024:
                                               idx0 - (half + 1) * 1024:-1])
                if prev is not None:
                    emit_pv(jt - 1, prev)
                prev = pt
            emit_pv(15, prev)
            # fast psum release: copy pv -> sbuf (split across ACT/DVE), then
            # run the (slow) normalization chain off the critical path.
            pvf = nrm.tile([65, T], F32, tag="pvf", name=f"pvf_{h}")
            nc.scalar.copy(pvf[:, 0:1024], pv[:, 0:1024])
            nc.vector.tensor_copy(pvf[:, 1024:T], pv[:, 1024:T])
            rsq = nrm.tile([128, 16], F32, tag="rsq", name=f"rsq_{h}")
            nc.sync.dma_start(rsq[:], pvf[64:65, :])
            rsr = nrm.tile([128, 16], F32, tag="rsr", name=f"rsr_{h}")
            nc.vector.reciprocal(rsr[:], rsq[:])
            rsf = nrm.tile([1, T], F32, tag="rsf", name=f"rsf_{h}")
            nc.sync.dma_start(rsf[:], rsr[:])
            rsb = nrm.tile([64, T], F32, tag="rsb", name=f"rsb_{h}")
            nc.gpsimd.partition_broadcast(rsb[:], rsf[:], channels=64)
            # normalized heads land in o2 [128, 2, T]: even head -> partitions
            # 0..63 directly; odd head -> staging tile then DMA to 64..127.
            if h % 2 == 0:
                nc.vector.tensor_mul(o2_sb[0:64, m, :], pvf[0:64, :], rsb[:])
            else:
                otmp = nrm.tile([64, T], BF16, tag="otmp", name=f"otmp_{h}")
                nc.vector.tensor_mul(otmp[:], pvf[0:64, :], rsb[:])
                nc.sync.dma_start(o2_sb[64:128, m, :], otmp[:])

    # ---- Phase 3: output projection (partial; host sums across head-groups) ----
    with tc.tile_pool(name="yps", bufs=3, space="PSUM") as yps, \
         tc.tile_pool(name="yo", bufs=2) as yo:
        for ic in range(16):
            for mt in range(2):
                ps = yps.tile([128, 512], F32, tag="y", name=f"yps_{ic}_{mt}")
                for m in range(2):
                    nc.tensor.matmul(ps[:], lhsT=o2_sb[:, m, ic * 128:(ic + 1) * 128],
                                     rhs=wo_sb[:, m, mt * 512:(mt + 1) * 512],
                                     start=(m == 0), stop=(m == 1))
                yt = yo.tile([128, 512], F32, tag="yt", name=f"yt_{ic}_{mt}")
                nc.scalar.copy(yt[:], ps[:])
                nc.sync.dma_start(y_d[ic * 128:(ic + 1) * 128, mt * 512:(mt + 1) * 512],
                                  yt[:])


_NC = None


def build_nc():
    global _NC
    if _NC is not None:
        return _NC
    nc = bacc.Bacc("TRN2", target_bir_lowering=False, debug=False, num_devices=8)
    xt_d = nc.dram_tensor("xt", [128, KC, T], BF16, kind="ExternalInput").ap()
    ct_d = nc.dram_tensor("ct", [128, KC, T], BF16, kind="ExternalInput").ap()
    wq_d = nc.dram_tensor("wq", [128, KC, 256], BF16, kind="ExternalInput").ap()
    wk_d = nc.dram_tensor("wk", [128, KC, 256], BF16, kind="ExternalInput").ap()
    wv_d = nc.dram_tensor("wv", [128, KC, 256], BF16, kind="ExternalInput").ap()
    wo_d = nc.dram_tensor("wo", [128, 2, D], BF16, kind="ExternalInput").ap()
    eb_d = nc.dram_tensor("eb", [128, HL, EBL], BF16, kind="ExternalInput").ap()
    y_d = nc.dram_tensor("y", [T, D], F32, kind="ExternalOutput").ap()

    with tile.TileContext(nc) as tc, ExitStack() as ctx:
        _build_kernel(ctx, tc, y_d, xt_d, ct_d, wq_d, wk_d, wv_d, wo_d, eb_d)
    nc.compile()
    _NC = nc
    return nc


def _to_chunked(mat_t, cols):
    """[D, cols] -> [128, KC, cols] with partition dim first."""
    return np.ascontiguousarray(
        mat_t.reshape(KC, 128, cols).transpose(1, 0, 2)).astype(bf16)


def make_in_maps(x, context, Wq, Wk, Wv, Wo):
    scale = np.float32(1.0 / np.sqrt(DH))
    # exp-bias skew tables per global head
    p = np.arange(128, dtype=np.int64)[:, None]
    t = np.arange(EBL, dtype=np.int64)[None, :]
    dist = np.abs(p + t - T0) // PERIOD          # [128, EBL]
    in_maps = []
    for c in range(8):
        b = c // 4
        h0 = (c % 4) * HL
        rows = slice(h0 * DH, (h0 + HL) * DH)
        xt = np.ascontiguousarray(
            x[b].T.reshape(KC, 128, T).transpose(1, 0, 2)).astype(bf16)
        ct = np.ascontiguousarray(
            context[b].T.reshape(KC, 128, T).transpose(1, 0, 2)).astype(bf16)
        wq = _to_chunked(np.ascontiguousarray((Wq[rows] * scale).T), 256)
        wk = _to_chunked(np.ascontiguousarray(Wk[rows].T), 256)
        wv = _to_chunked(np.ascontiguousarray(Wv[rows].T), 256)
        wo = np.ascontiguousarray(
            Wo[:, rows].T.reshape(2, 128, D).transpose(1, 0, 2)).astype(bf16)
        eb = np.empty((128, HL, EBL), dtype=bf16)
        for hl in range(HL):
            hs = 2.0 ** (-(h0 + hl + 1))
            eb[:, hl, :] = np.exp(-hs * dist).astype(bf16)
        in_maps.append({"xt": xt, "ct": ct, "wq": wq, "wk": wk, "wv": wv,
                        "wo": wo, "eb": np.ascontiguousarray(eb)})
    return in_maps


def kernel(x, context, Wq, Wk, Wv, Wo, bo, _collect=None):
    x = np.asarray(x, dtype=np.float32)
    context = np.asarray(context, dtype=np.float32)
    Wq = np.asarray(Wq, dtype=np.float32)
    Wk = np.asarray(Wk, dtype=np.float32)
    Wv = np.asarray(Wv, dtype=np.float32)
    Wo = np.asarray(Wo, dtype=np.float32)
    bo = np.asarray(bo, dtype=np.float32)

    nc = build_nc()
    in_maps = make_in_maps(x, context, Wq, Wk, Wv, Wo)
    res = run_bass_kernel_spmd(nc, in_maps, list(range(8)))
    if _collect is not None:
        _collect.append(res)

    out = np.empty((B, T, D), dtype=np.float32)
    for b in range(2):
        acc = res.results[4 * b]["y"].astype(np.float32).copy()
        for c in range(4 * b + 1, 4 * b + 4):
            acc += res.results[c]["y"]
        out[b] = acc + bo[None, :]
    return out
